# revision 48
# baseline (speedup 1.0000x reference)
"""Trainium2 Bass kernel for the nn_Decoder LSTM-decoder problem.

Reference computation (per agent, 12 steps):
    gates = dec_in @ w_ih.T + h @ w_hh.T + (b_ih + b_hh)
    i, f, g, o = split(gates); c = sig(f)*c + sig(i)*tanh(g); h = sig(o)*tanh(c)
    rel = h @ w_hp.T + b_hp; dec_in = rel @ w_se.T + b_se
Output: rel per step, [12, N, 2].

Key algebraic fusion: dec_in_t is a linear function of h_t, so for steps >= 2
    gates_t = h_{t-1} @ W_eff.T + b_eff,  W_eff = w_hh + w_ih @ w_se @ w_hp
and step 1 uses w_hh plus U = w_ih @ w_se applied to last_pos_rel.
last_pos is dead (never affects the output).

Distribution: pure data parallel over the agent axis, 8192 agents per core
on 8 NeuronCores; weights replicated.

On-chip layout: [feature partitions, agent free]. h0/c0/last_pos_rel are
transposed to [feature, agent] bf16 on the host so they DMA straight into
the state tiles (no on-device transposes or casts). Agents are processed in
1024-agent pairs (one [128, 1024] PSUM tile per gate); ACT does the four
gate activations per pair plus one paired tanh(c) over 2048 agents. The
rel matmuls for all 8 pairs of a step write one [16, 1024] PSUM tile
(pair p at partitions 2p, 2p+1), so a single tensor_scalar_add per step
applies b_hp and stages to SBUF; the per-step output is then re-blocked
via SBUF->SBUF DMA and pair-interleaved on DVE so the final DRAM write has
512-byte contiguous runs spread across all 16 DMA ports.
"""

import sys

if "/opt/trn_rl_repo" not in sys.path:
    sys.path.insert(0, "/opt/trn_rl_repo")

import numpy as np

T = 12          # steps
H = 128         # hidden dim
NCORES = 8
NPC = 8192      # agents per core
CH = 512        # agents per chunk (one PSUM bank at fp32)
PAIR = 2 * CH   # agents per gate-tile

_CACHE = {}


def _build_program(npc):
    import concourse.bass as bass
    import concourse.tile as tile
    from concourse import bacc, mybir

    dt = mybir.dt
    f32 = dt.float32
    bf16 = dt.bfloat16
    Act = mybir.ActivationFunctionType

    npair = npc // PAIR
    assert npc % (2 * PAIR) == 0 and npc >= 2 * PAIR
    nblk = npc // 64   # output partition blocks (64 agents each)

    nc = bacc.Bacc(
        "TRN2",
        target_bir_lowering=False,
        debug=False,
        num_devices=NCORES,
    )

    def din(name, shape, dt_=None):
        return nc.dram_tensor(
            name, list(shape), dt_ or f32, kind="ExternalInput"
        ).ap()

    # Host pre-transposed state, [feature, agent] bf16.
    h0_d = din("h0t", [H, npc], bf16)
    c0_d = din("c0t", [H, npc], bf16)
    lpr_d = din("lprt", [2, npc], bf16)
    # lhsT layouts, K on partitions. Gate order [i, f, o, g].
    # K/M dims below 65 are zero-padded to 65 so every matmul infers the
    # same (128, 128) PE tile size -- a tile-mode change drains TensorE.
    wg_d = din("wg", [H, 4 * H], bf16)    # W_eff.T columns gate-ordered
    whh_d = din("whh", [H, 4 * H], bf16)  # w_hh.T (step 1)
    u_d = din("u", [65, 4 * H], bf16)     # (w_ih @ w_se).T zero-padded
    bias_d = din("bias", [H, 8])          # ACT bias: [b_eff | b1] x [i,f,o,g]
    # Two rel-weight pads put even/odd pairs' outputs at PSUM partition
    # rows (96,97)/(64,65) of one shared tile: one bias-add per 2 pairs.
    whpa_d = din("whpa", [H, 98], bf16)   # w_hp.T at cols 96:98
    whpb_d = din("whpb", [H, 66], bf16)   # w_hp.T at cols 64:66
    bhp_d = din("bhp", [128, 1])          # b_hp striped (x at 32k, y at 32k+1)
    out_d = nc.dram_tensor("out", [T, npc, 2], f32, kind="ExternalOutput").ap()

    with tile.TileContext(nc) as tc:
        with (
            tc.tile_pool(name="wpool", bufs=1) as wp,
            tc.tile_pool(name="state", bufs=1) as state,
            tc.tile_pool(name="sig", bufs=4) as sigp,
            tc.tile_pool(name="tmp", bufs=4) as tmpp,
            tc.tile_pool(name="outp", bufs=3) as outp,
            tc.tile_pool(name="ps", bufs=3, space="PSUM") as psp,
            tc.tile_pool(name="psr", bufs=1, space="PSUM") as psr,
        ):
            def wtile(ap, shape, tag, dt_=None):
                t_ = wp.tile(list(shape), dt_ or f32, tag=tag)
                nc.sync.dma_start(t_[:], ap)
                return t_

            h_sb = state.tile([H, npc], bf16, tag="h")
            c_sb = state.tile([H, npc], bf16, tag="c")
            lpr_t = state.tile([65, npc], bf16, tag="lpr")

            def preload_pair(p):
                cols = slice(p * PAIR, (p + 1) * PAIR)
                # per-pair zero of the K-pad rows (a whole-tile memset
                # would gate the first u-matmul by ~7us)
                nc.vector.memset(lpr_t[:, cols], 0.0)
                nc.sync.dma_start(lpr_t[0:2, cols], lpr_d[:, cols])
                nc.sync.dma_start(h_sb[:, cols], h0_d[:, cols])
                nc.sync.dma_start(c_sb[:, cols], c0_d[:, cols])

            # DMA issue order follows first-use: step-0 weights and pair 0
            # state first, wg (needed from step 1) last.
            u = wtile(u_d, [65, 4 * H], "u", bf16)
            whh = wtile(whh_d, [H, 4 * H], "whh", bf16)
            bias = wtile(bias_d, [H, 8], "bias")
            preload_pair(0)
            whpa = wtile(whpa_d, [H, 98], "whpa", bf16)
            whpb = wtile(whpb_d, [H, 66], "whpb", bf16)
            bhp = wtile(bhp_d, [128, 1], "bhp")
            preload_pair(1)
            wg = wtile(wg_d, [H, 4 * H], "wg", bf16)

            def front(t, p):
                """Gates + sigma_i/sigma_f/tanh_g + m1/m2 + sigma_o + c-add.
                Returns so (needed by the deferred h update)."""
                first = t == 0
                W = whh if first else wg
                bcol = 4 if first else 0
                cols = slice(p * PAIR, (p + 1) * PAIR)
                c_pr = c_sb[:, cols]
                gt = {}
                # allocation order matches ACT consumption order:
                # wg column groups are [i, f, o, g] -> alloc i(0), f(1), g(3), o(2)
                for g in (0, 1, 3, 2):
                    gt[g] = psp.tile([128, PAIR], f32, tag="ps", name=f"gt{g}")
                    wsl = slice(g * H, (g + 1) * H)
                    # matmul output is capped at one PSUM bank (512 fp32)
                    for half in range(2):
                        hs = slice((p * 2 + half) * CH,
                                   (p * 2 + half + 1) * CH)
                        osl = slice(half * CH, (half + 1) * CH)
                        if first:
                            nc.tensor.matmul(
                                gt[g][:, osl], u[:, wsl], lpr_t[:, hs],
                                start=True, stop=False)
                        nc.tensor.matmul(
                            gt[g][:, osl], W[:, wsl], h_sb[:, hs],
                            start=not first, stop=True)

                si = sigp.tile([128, PAIR], bf16, tag="si")
                sf = sigp.tile([128, PAIR], bf16, tag="sf")
                tg = sigp.tile([128, PAIR], bf16, tag="tg")
                nc.scalar.activation(si[:], gt[0][:], Act.Sigmoid,
                                     bias=bias[:, bcol:bcol + 1])
                nc.scalar.activation(sf[:], gt[1][:], Act.Sigmoid,
                                     bias=bias[:, bcol + 1:bcol + 2])
                nc.scalar.activation(tg[:], gt[3][:], Act.Tanh,
                                     bias=bias[:, bcol + 3:bcol + 4])
                m1 = tmpp.tile([128, PAIR], bf16, tag="m1")
                nc.vector.tensor_mul(m1[:], sf[:], c_pr)
                m2 = tmpp.tile([128, PAIR], bf16, tag="m2")
                nc.vector.tensor_mul(m2[:], si[:], tg[:])
                # so halves of consecutive pairs share one [128, 2*PAIR]
                # tile so the paired h update is a single 2048-wide op.
                if p % 2 == 0:
                    so2 = sigp.tile([128, 2 * PAIR], bf16, tag="so2",
                                    name=f"so2_{t}_{p // 2}", bufs=3)
                    front.so2 = so2
                else:
                    so2 = front.so2
                half = p % 2
                nc.scalar.activation(so2[:, half * PAIR:(half + 1) * PAIR],
                                     gt[2][:], Act.Sigmoid,
                                     bias=bias[:, bcol + 2:bcol + 3])
                nc.vector.tensor_add(c_pr, m1[:], m2[:])
                return so2

            def back2(t, g, so2):
                """tanh(c) + h update over a 2-pair group (2048 agents)."""
                cols = slice(g * 2 * PAIR, (g + 1) * 2 * PAIR)
                tcl = sigp.tile([128, 2 * PAIR], bf16, tag="tc", bufs=2)
                nc.scalar.activation(tcl[:], c_sb[:, cols], Act.Tanh)
                nc.vector.tensor_mul(h_sb[:, cols], so2[:], tcl[:])

            def rel_pair(t, p, blks):
                """rel = w_hp @ h + b_hp (deferred four units). Even pair
                lands at rows 96:98, odd pair at rows 64:66 of a shared
                PSUM tile (via zero-padded weights, keeping the (128,128)
                PE tile), so one bias-add covers two pairs. x/y re-blocked
                via SBUF->SBUF DMA."""
                xblk, yblk, _ = blks
                even = p % 2 == 0
                if even:
                    rel_pair.rp = psr.tile([98, PAIR], f32, tag="rel",
                                           name=f"rp{t}_{p // 2}")
                rp = rel_pair.rp
                W, rows = (whpa, 98) if even else (whpb, 66)
                for half in range(2):
                    hs = slice((p * 2 + half) * CH,
                               (p * 2 + half + 1) * CH)
                    osl = slice(half * CH, (half + 1) * CH)
                    nc.tensor.matmul(
                        rp[0:rows, osl], W[:], h_sb[:, hs],
                        start=True, stop=True)
                if not even:
                    ex = tmpp.tile([98, PAIR], f32, tag="ex", bufs=2)
                    nc.vector.tensor_scalar_add(ex[:], rp[:], bhp[0:98, 0:1])
                    pe, po = slice(16 * (p - 1), 16 * p), slice(
                        16 * p, 16 * (p + 1))
                    nc.sync.dma_start(xblk[pe, :], ex[96:97, :])
                    nc.sync.dma_start(yblk[pe, :], ex[97:98, :])
                    nc.sync.dma_start(xblk[po, :], ex[64:65, :])
                    nc.sync.dma_start(yblk[po, :], ex[65:66, :])
                    # interleave + write out this quarter-step right away
                    # (keeps the final-step tail short)
                    relpk = blks[2]
                    rows = slice(16 * (p - 1), 16 * (p + 1))
                    rv = relpk[:].rearrange("q (a k) -> q a k", k=2)
                    nc.vector.tensor_copy(rv[rows, :, 0], xblk[rows, :])
                    nc.vector.tensor_copy(rv[rows, :, 1], yblk[rows, :])
                    nc.sync.dma_start(
                        out_d[t].rearrange("(q a) k -> q (a k)", a=64)[rows],
                        relpk[rows, :])

            # ---- unit pipeline: FRONT(k) | BACK2 | REL(k-3) ----
            units = [(t, p) for t in range(T) for p in range(npair)]
            pend_back = []   # (t, group, so2)
            pend_rel = []    # (t, p)
            blks = {}
            done_pairs = {t: 0 for t in range(T)}

            def emit_rel(t, p):
                rel_pair(t, p, blks[t])
                done_pairs[t] += 1
                if done_pairs[t] == npair:
                    blks.pop(t)

            for k, (t, p) in enumerate(units):
                if t not in blks:
                    xb = outp.tile([nblk, 64], f32, tag="xblk",
                                   name=f"xb{t}")
                    yb = outp.tile([nblk, 64], f32, tag="yblk",
                                   name=f"yb{t}")
                    rpk = outp.tile([nblk, 128], f32, tag="relpk",
                                    name=f"rpk{t}")
                    blks[t] = (xb, yb, rpk)
                if t == 0 and p + 2 < npair:
                    preload_pair(p + 2)
                so2 = front(t, p)
                if len(pend_back) >= 1:
                    back2(*pend_back.pop(0))
                if p % 2 == 1:
                    pend_back.append((t, p // 2, so2))
                if len(pend_rel) >= 3:
                    emit_rel(*pend_rel.pop(0))
                pend_rel.append((t, p))
            while pend_back:
                back2(*pend_back.pop(0))
            while pend_rel:
                emit_rel(*pend_rel.pop(0))

    nc.compile()
    return nc


def _fold_weights(w_ih, w_hh, b_ih, b_hh, w_se, b_se, w_hp, b_hp):
    """Host-side constant folding. Gate order [i, f, o, g] (torch order in
    the 4H rows is i, f, g, o)."""
    import ml_dtypes
    mf = ml_dtypes.bfloat16
    perm = np.concatenate([
        np.arange(0, H), np.arange(H, 2 * H),
        np.arange(3 * H, 4 * H), np.arange(2 * H, 3 * H),
    ])
    W_eff = w_hh + w_ih @ w_se @ w_hp                      # [4H, H]
    b_eff = (b_hp @ w_se.T + b_se) @ w_ih.T + b_ih + b_hh  # [4H]
    U = w_ih @ w_se                                        # [4H, 2]
    b1 = b_se @ w_ih.T + b_ih + b_hh                       # [4H]
    bhp_pat = np.zeros((128, 1), np.float32)
    bhp_pat[0::32, 0] = b_hp[0]
    bhp_pat[1::32, 0] = b_hp[1]

    Wp, bp = W_eff[perm], b_eff[perm]
    Whhp, Up, b1p = w_hh[perm], U[perm], b1[perm]
    f = np.float32
    bias = np.stack([bp[0:H], bp[H:2*H], bp[2*H:3*H], bp[3*H:4*H],
                     b1p[0:H], b1p[H:2*H], b1p[2*H:3*H], b1p[3*H:4*H]],
                    axis=1)  # [H, 8]
    # Zero-pad K/M so every matmul shares the gate matmuls' (128, 128)
    # PE tile size (tile-mode changes drain TensorE). The two rel pads
    # also place even/odd pairs at distinct PSUM partition rows.
    u_pad = np.zeros((65, 4 * H), np.float32)
    u_pad[0:2] = Up.T
    whpa = np.zeros((H, 98), np.float32)
    whpa[:, 96:98] = w_hp.T
    whpb = np.zeros((H, 66), np.float32)
    whpb[:, 64:66] = w_hp.T
    return {
        "wg": np.ascontiguousarray(Wp.T.astype(mf)),
        "whh": np.ascontiguousarray(Whhp.T.astype(mf)),
        "u": np.ascontiguousarray(u_pad.astype(mf)),
        "bias": np.ascontiguousarray(bias, f),
        "whpa": np.ascontiguousarray(whpa.astype(mf)),
        "whpb": np.ascontiguousarray(whpb.astype(mf)),
        "bhp": bhp_pat,
    }


def kernel(last_pos, last_pos_rel, h0, c0,
           w_ih, w_hh, b_ih, b_hh, w_se, b_se, w_hp, b_hp):
    import ml_dtypes
    mf = ml_dtypes.bfloat16
    # Host-side transpose to the on-chip [feature, agent] layout.
    lprT = np.ascontiguousarray(
        np.asarray(last_pos_rel, np.float32).T.astype(mf))
    h0T = np.ascontiguousarray(np.asarray(h0, np.float32).T.astype(mf))
    c0T = np.ascontiguousarray(np.asarray(c0, np.float32).T.astype(mf))
    consts = _fold_weights(
        np.asarray(w_ih, np.float32), np.asarray(w_hh, np.float32),
        np.asarray(b_ih, np.float32), np.asarray(b_hh, np.float32),
        np.asarray(w_se, np.float32), np.asarray(b_se, np.float32),
        np.asarray(w_hp, np.float32), np.asarray(b_hp, np.float32),
    )

    npeds = h0.shape[0]
    npc = npeds // NCORES
    if "nc" not in _CACHE or _CACHE.get("npc") != npc:
        _CACHE["nc"] = _build_program(npc)
        _CACHE["npc"] = npc
    nc = _CACHE["nc"]

    in_maps = []
    for ci in range(NCORES):
        cols = slice(ci * npc, (ci + 1) * npc)
        m = {
            "h0t": np.ascontiguousarray(h0T[:, cols]),
            "c0t": np.ascontiguousarray(c0T[:, cols]),
            "lprt": np.ascontiguousarray(lprT[:, cols]),
        }
        m.update(consts)
        in_maps.append(m)

    from concourse.bass_utils import run_bass_kernel_spmd
    import os

    res = run_bass_kernel_spmd(
        nc, in_maps, list(range(NCORES)),
        tmpdir=os.environ.get("KERNEL_TRACE_DIR"),
    )
    _CACHE["exec_time_ns"] = res.exec_time_ns
    _CACHE["results"] = res
    outs = [np.asarray(res.results[i]["out"]) for i in range(NCORES)]
    return np.concatenate(outs, axis=1)
